# revision 1
# baseline (speedup 1.0000x reference)
"""Trainium2 Bass kernel for nn_Decoder_9045201125559.

Computes, for B=32 batch rows and T=128 timesteps:
    x      = emb[dst[:, :T]]                          [B,T,E]
    gates  = x @ W_ih.T + h0 @ W_hh.T + b_ih + b_hh   [B,T,4H]
    i,f,g,o = split(gates); i,f,o=sigmoid; g=tanh
    c      = f*c0 + i*g ; h = o*tanh(c)               [B,T,H]
    logits = h @ fc_w.T + fc_b                        [B,T,V]

Sharding over 8 NeuronCores:
  - phase A: data-parallel over batch (4 rows / 512 tokens per core),
    computing h^T [H, 512] in transposed layout via one merged matmul
    (contraction over E+H=1536: embeddings stacked with per-token
    replicated h0 columns).
  - AllGather h^T (bf16, 1MB per rank) so every core holds all 4096
    token columns.
  - phase C: vocab-parallel logits matmul; each core computes
    [4096 tokens, 4000 vocab] with fc_w^T resident in SBUF.
"""

import sys

sys.path.insert(0, "/opt/trn_rl_repo")

import numpy as np
import ml_dtypes

from concourse import bacc
import concourse.mybir as mybir
import concourse.tile as tile
from concourse.bass_utils import run_bass_kernel_spmd

BF16 = ml_dtypes.bfloat16

V, E, H = 32000, 512, 1024
B, T = 32, 128
NCORES = 8
BL = B // NCORES          # 4 local batch rows per core
TL = BL * T               # 512 local tokens per core
TT = B * T                # 4096 total tokens
VS = V // NCORES          # 4000 vocab columns per core
VP = 4096                 # padded vocab (8 n-blocks of 512)
KE = E // 128             # 4 contraction chunks for the gates matmul
KH = H // 128             # 8 contraction chunks for the logits matmul
MG = (4 * H) // 128       # 32 gate-row tiles
NB = VP // 512            # 8 psum n-blocks
MT = TT // 128            # 32 token tiles

_nc = None


def _build():
    nc = bacc.Bacc("TRN2", num_devices=NCORES, target_bir_lowering=False)
    f32 = mybir.dt.float32
    bf16 = mybir.dt.bfloat16

    # ---- per-core DRAM I/O ----
    xt_d = nc.dram_tensor("xt", [128, KE, TL], bf16, kind="ExternalInput")
    wih_d = nc.dram_tensor("wih", [128, MG, KE, 128], bf16, kind="ExternalInput")
    # fullbias[p, mg*BL+b] = (h0 @ W_hh.T + b_ih + b_hh)[4*core+b, mg*128+p]
    fbias_d = nc.dram_tensor("fbias", [128, MG * BL], f32, kind="ExternalInput")
    c0t_d = nc.dram_tensor("c0t", [128, KH * BL], f32, kind="ExternalInput")
    fcw_d = nc.dram_tensor("fcw", [128, KH, VP], bf16, kind="ExternalInput")
    fcb_d = nc.dram_tensor("fcb", [128, VP], f32, kind="ExternalInput")
    out_d = nc.dram_tensor("out", [TT, VS], f32, kind="ExternalOutput")

    # AllGather buffers (internal DRAM; output must be Shared)
    hag_in = nc.dram_tensor("hag_in", [H, TL], bf16, kind="Internal")
    hag_out = nc.dram_tensor("hag_out", [NCORES * H, TL], bf16,
                             kind="Internal", addr_space="Shared")

    Sig = mybir.ActivationFunctionType.Sigmoid
    Tanh = mybir.ActivationFunctionType.Tanh

    with tile.TileContext(nc) as tc:
        with tc.tile_pool(name="const", bufs=1) as const:
            # resident for the whole kernel (DMAs emitted after phase A so the
            # phase-A-critical loads go out first)
            fcw_sb = const.tile([128, KH, VP], bf16)
            fcb_sb = const.tile([128, VP], f32)

            # ---------------- phase A ----------------
            with tc.tile_pool(name="pa", bufs=1) as pa, \
                 tc.tile_pool(name="pa_w", bufs=3) as pa_w, \
                 tc.tile_pool(name="pa_act", bufs=2) as pa_act, \
                 tc.tile_pool(name="pa_tmp", bufs=3) as pa_tmp, \
                 tc.tile_pool(name="pa_ps", bufs=8, space="PSUM") as pa_ps:

                xt_sb = pa.tile([128, KE, TL], bf16)
                fbias_sb = pa.tile([128, MG * BL], f32)
                c0t_sb = pa.tile([128, KH * BL], f32)
                ht_sb = pa.tile([128, KH, TL], bf16)
                nc.sync.dma_start(xt_sb[:], xt_d[:])
                nc.sync.dma_start(fbias_sb[:], fbias_d[:])
                nc.sync.dma_start(c0t_sb[:], c0t_d[:])

                for hc in range(KH):
                    acts = []
                    for gate in range(4):  # i, f, g, o
                        mg = gate * KH + hc
                        wc_sb = pa_w.tile([128, KE, 128], bf16, tag="wc")
                        nc.sync.dma_start(wc_sb[:], wih_d[:, mg])
                        ps = pa_ps.tile([128, TL], mybir.dt.float32, tag="psA")
                        for kc in range(KE):
                            nc.tensor.matmul(ps[:], wc_sb[:, kc], xt_sb[:, kc],
                                             start=(kc == 0), stop=(kc == KE - 1))
                        act = pa_act.tile([128, TL], f32, tag=f"act{gate}")
                        for b in range(BL):
                            nc.scalar.activation(
                                act[:, b * T:(b + 1) * T],
                                ps[:, b * T:(b + 1) * T],
                                Tanh if gate == 2 else Sig,
                                bias=fbias_sb[:, mg * BL + b:mg * BL + b + 1])
                        acts.append(act)
                    i_t, f_t, g_t, o_t = acts
                    # c = f*c0 + i*g ; h = o*tanh(c)
                    c_sb = pa_tmp.tile([128, TL], f32, tag="c")
                    for b in range(BL):
                        s = slice(b * T, (b + 1) * T)
                        nc.vector.tensor_scalar_mul(
                            c_sb[:, s], f_t[:, s],
                            c0t_sb[:, hc * BL + b:hc * BL + b + 1])
                    ig_sb = pa_tmp.tile([128, TL], f32, tag="ig")
                    nc.vector.tensor_mul(out=ig_sb[:], in0=i_t[:], in1=g_t[:])
                    nc.vector.tensor_add(out=c_sb[:], in0=c_sb[:], in1=ig_sb[:])
                    tc_sb = pa_tmp.tile([128, TL], f32, tag="tc")
                    nc.scalar.activation(tc_sb[:], c_sb[:], Tanh)
                    nc.vector.tensor_mul(out=ht_sb[:, hc], in0=o_t[:], in1=tc_sb[:])

                # local h^T -> DRAM for the collective
                nc.sync.dma_start(
                    hag_in.rearrange("(kc p) t -> p kc t", p=128), ht_sb[:])

                # phase-C weights: emitted last and split into ~1MB chunks so
                # the phase-A streaming loads interleave on the DMA engines
                for kc in range(KH):
                    nc.sync.dma_start(fcw_sb[:, kc], fcw_d[:, kc])
                for q in range(4):
                    nc.sync.dma_start(fcb_sb[:, q * 1024:(q + 1) * 1024],
                                      fcb_d[:, q * 1024:(q + 1) * 1024])

            nc.gpsimd.collective_compute(
                "AllGather",
                mybir.AluOpType.bypass,
                replica_groups=[list(range(NCORES))],
                ins=[hag_in[:]],
                outs=[hag_out[:]],
            )

            # ---------------- phase C ----------------
            with tc.tile_pool(name="pc", bufs=1) as pc, \
                 tc.tile_pool(name="pc_out", bufs=2) as pc_out, \
                 tc.tile_pool(name="pc_ps", bufs=8, space="PSUM") as pc_ps:

                htall_sb = pc.tile([128, KH, TT], bf16)
                for r in range(NCORES):
                    nc.sync.dma_start(
                        htall_sb[:, :, r * TL:(r + 1) * TL],
                        hag_out[r * H:(r + 1) * H, :].rearrange(
                            "(kc p) t -> p kc t", p=128))

                nw = [512] * 7 + [VS - 7 * 512]  # last n-block unpadded (416)
                for m in range(MT):
                    ms = slice(m * 128, (m + 1) * 128)
                    stage = pc_out.tile([128, VP], f32, tag="stage")
                    for half in range(2):
                        nblk = range(half * 4, half * 4 + 4)
                        pss = {n: pc_ps.tile([128, 512], mybir.dt.float32,
                                             tag="psC", name=f"psC{n}")
                               for n in nblk}
                        for kc in range(KH):
                            lhsT = htall_sb[:, kc, ms]
                            for n in nblk:
                                nc.tensor.matmul(
                                    pss[n][:, :nw[n]], lhsT,
                                    fcw_sb[:, kc, n * 512:n * 512 + nw[n]],
                                    start=(kc == 0), stop=(kc == KH - 1))
                        for n in nblk:
                            nc.vector.tensor_add(
                                out=stage[:, n * 512:n * 512 + nw[n]],
                                in0=pss[n][:, :nw[n]],
                                in1=fcb_sb[:, n * 512:n * 512 + nw[n]])
                    nc.sync.dma_start(out_d[ms, :], stage[:, :VS])

    nc.compile()
    return nc


def _get_nc():
    global _nc
    if _nc is None:
        _nc = _build()
    return _nc


def _prep_inputs(dst, h0, c0, emb, W_ih, W_hh, b_ih, b_hh, fc_w, fc_b):
    dst = np.asarray(dst)[:, :T]
    h0 = np.asarray(h0, dtype=np.float32)
    c0 = np.asarray(c0, dtype=np.float32)
    emb_bf = np.asarray(emb, dtype=np.float32).astype(BF16)
    W_ih = np.asarray(W_ih, np.float32)
    # wih layout [p, mg, kc, mi] = W_ih[mg*128+mi, kc*128+p]
    wih = np.ascontiguousarray(
        W_ih.astype(BF16).T.reshape(KE, 128, MG, 128).transpose(1, 2, 0, 3))
    # recurrent contribution is tiny (0.27 GFLOP total) and identical for
    # every timestep -> fold into the per-(gate-row, batch) activation bias
    base = (h0 @ np.asarray(W_hh, np.float32).T
            + np.asarray(b_ih, np.float32) + np.asarray(b_hh, np.float32))  # [B, 4H]

    fc_w = np.asarray(fc_w, np.float32)
    fc_b = np.asarray(fc_b, np.float32)

    in_maps = []
    for ci in range(NCORES):
        rows = slice(ci * BL, (ci + 1) * BL)
        x = emb_bf[dst[rows]]                      # [BL, T, E] bf16
        xT = x.reshape(TL, E).T.astype(BF16)       # [E, TL]
        xt = np.ascontiguousarray(
            xT.reshape(KE, 128, TL).transpose(1, 0, 2))          # [p, kc, t]

        # fbias[p, mg*BL+b] = base[4ci+b, mg*128+p]
        fbias = np.ascontiguousarray(
            base[rows].T.reshape(MG, 128, BL).transpose(1, 0, 2).reshape(128, MG * BL))
        c0t = np.ascontiguousarray(
            c0[rows].T.reshape(KH, 128, BL).transpose(1, 0, 2).reshape(128, KH * BL))

        vsl = slice(ci * VS, (ci + 1) * VS)
        fcwT = np.zeros((VP, H), np.float32)
        fcwT[:VS] = fc_w[vsl]
        fcw = np.ascontiguousarray(
            fcwT.T.astype(BF16).reshape(KH, 128, VP).transpose(1, 0, 2))
        fcb = np.zeros((VP,), np.float32)
        fcb[:VS] = fc_b[vsl]
        fcb = np.ascontiguousarray(np.broadcast_to(fcb, (128, VP)))

        in_maps.append({
            "xt": xt, "wih": wih, "fbias": fbias, "c0t": c0t,
            "fcw": fcw, "fcb": fcb,
        })
    return in_maps


def _run(inputs: dict, trace: bool = False):
    nc = _get_nc()
    in_maps = _prep_inputs(**inputs)
    res = run_bass_kernel_spmd(nc, in_maps, core_ids=list(range(NCORES)),
                               trace=trace)
    logits = np.concatenate(
        [res.results[ci]["out"].reshape(B, T, VS) for ci in range(NCORES)],
        axis=2)
    return logits, res


def kernel(**inputs):
    logits, _ = _run(inputs, trace=False)
    return logits

